# revision 20
# baseline (speedup 1.0000x reference)
"""LookAheadMask kernel for Trainium2.

out[b, r, c] = 1.0 if c > r else x[b, r, c], for x of shape (8, 4096, 4096) f32.

Sharding: batch dim across 8 NeuronCores (data parallel, no communication).

The op is an in-place masked_fill: out == x everywhere except the strictly
upper triangle, which is constant 1.0.  The PJRT launch path donates a
host-staged buffer as the kernel's output tensor (the stock runner stages
zeros and kernels rely on that zero-init); here we stage x itself, so the
device kernel only writes the masked region instead of first copying the
31 MiB lower triangle through HBM twice (DRAM->DRAM read+write).  Per-core
HBM traffic drops from ~99 MiB to ~33 MiB, which matters because the
measured baseline was HBM-bandwidth-bound (both HWDGE queues >90% busy at
a combined ~324 GB/s, right at the per-core HBM share).

Device-side plan per core (S=4096, P=128), built from measured DMA-queue
behavior: every queue is packet-slot-bound (~30-40 ns/packet regardless of
packet size up to 8 KiB), SWDGE packs sub-4KiB descriptors into 4 KiB
packets while HWDGE does not, SWDGE also emits ~0.45 tiny bookkeeping
packets per descriptor, and the SDMA engines split packet slots roughly
equally across active queues:

  - strict-upper staircase as a quad-tree of same-size square rects;
    all rects of width w sit at constant stride 2w(S+1) so each level is
    one 3D-strided DMA from an SBUF ones tile.  The 2048/1024 levels and
    half the 512 level (8/4/2 KiB descriptors) are split across the two
    HWDGE queues; the other 512-level rects (rewritten in 1 KiB-descriptor
    column halves) and the 256-level ride SWDGE, which packs sub-4KiB
    descriptors into 4 KiB packets.  SDMA packet slots are shared about
    equally across active queues, so per-queue packet counts are
    equalized (~2700 each).
  - near-diagonal triangles via a "shifted band" write on SWDGE:
    out[r, r+1 : r+257) = 1 for r < 3840 (one 2-dim AP with stride S+1).
    Everything right of the diagonal is 1, so spilling into the
    staircase is a harmless same-value overlap; no gather of x is needed
    (the old gather/scatter diagonal path cost ~120us of queue time at
    sub-1KiB descriptor rates).  The band also replaces the 128-level
    for those rows.  Rows 3840..3967 get a 128-wide band + the last
    128-level rect.
  - the final 128x128 diagonal block (rows 3968+, where a band would
    overrun the row end): 64 KiB gather -> gpsimd affine_select (keep x
    at/below the diagonal, 1.0 strictly above) -> 64 KiB scatter.
"""

import glob
import os
import tempfile

import numpy as np

from concourse import bass, mybir

S = 4096
P = 128
N_CORES = 8

_cached_nc = None


def _build():
    global _cached_nc
    if _cached_nc is not None:
        return _cached_nc

    nc = bass.Bass()
    out = nc.dram_tensor("out", [S, S], mybir.dt.float32, kind="ExternalOutput")

    # ones tile: 128 x 4096 f32 (2 MiB) -- every DMA below sources at most
    # 512K elements from it (element order is irrelevant: all are 1.0).
    F = 4096

    with (
        nc.Block() as block,
        nc.semaphore("dsem") as dsem,  # bulk ones/band DMA completions
        nc.semaphore("gsem") as gsem,  # corner gather done
        nc.semaphore("ssem") as ssem,  # corner scatter done
        nc.semaphore("msem") as msem,  # ones memset done
        nc.sbuf_tensor("ones", [P, F], mybir.dt.float32) as ones,
        nc.sbuf_tensor("corner_in", [P, P], mybir.dt.float32) as corner_in,
        nc.sbuf_tensor("corner_out", [P, P], mybir.dt.float32) as corner_out,
    ):
        # Quad-tree staircase level (width w, 2048//w rects, rect k at rows
        # [2wk, 2wk+w) x cols [2wk+w, 2wk+2w)), restricted to rects
        # [k0, k0+c).  All DMAs source <= 512K elements of the ones tile.
        def ones_level(eng, w, k0, c):
            return eng.dma_start(
                out=bass.AP(
                    out, w + k0 * 2 * w * (S + 1), [[2 * w * (S + 1), c], [S, w], [1, w]]
                ),
                in_=ones[:, : c * w * w // P],
            ).then_inc(dsem, 16)

        # 512-row slice of the 2048-level rect (rows[r0:r0+512] x
        # cols[2048:4096] halved into 256-row pieces for the 512K source
        # cap); 8 KiB descriptors.
        def ones_2048_quarter(eng, r0):
            for r in (r0, r0 + 256):
                eng.dma_start(
                    out=bass.AP(out, r * S + 2048, [[S, 256], [1, 2048]]),
                    in_=ones[:, :F],
                ).then_inc(dsem, 16)

        # Half (512 rows) of a 1024-level rect; 4 KiB descriptors.
        def ones_1024_half(eng, k, r0):
            eng.dma_start(
                out=bass.AP(
                    out, 1024 + k * 2048 * (S + 1) + r0 * S, [[S, 512], [1, 1024]]
                ),
                in_=ones[:, :F],
            ).then_inc(dsem, 16)

        # Corner: the last 128x128 diagonal block, rows/cols [3968:4096).
        corner_dram = bass.AP(out, 3968 * S + 3968, [[S, P], [1, P]])

        @block.vector
        def _(vector: bass.BassVectorEngine):
            # Split so the 512-level DMAs (which source only the first
            # 2048 cols) can start one memset earlier.
            vector.memset(ones[:, :2048], 1.0).then_inc(msem, 1)
            vector.memset(ones[:, 2048:], 1.0).then_inc(msem, 1)

        # Shifted-band write: out[r, r+1 : r+1+w) = 1 for nrows rows from
        # r0, covering the near-diagonal triangles; right of the diagonal
        # everything is 1, so spilling into the staircase region is a
        # harmless same-value overlap.
        def band(eng, r0, nrows, w):
            eng.dma_start(
                out=bass.AP(out, r0 * (S + 1) + 1, [[S + 1, nrows], [1, w]]),
                in_=ones[:, : nrows * w // P],
            ).then_inc(dsem, 16)

        @block.sync
        def _(sync: bass.BassEngine):
            sync.wait_ge(msem, 1)
            ones_level(sync, 512, 0, 1)
            sync.wait_ge(msem, 2)
            ones_1024_half(sync, 0, 0)
            ones_1024_half(sync, 0, 512)
            ones_2048_quarter(sync, 0)
            ones_2048_quarter(sync, 512)
            sync.wait_ge(dsem, 16 * 23)
            sync.wait_ge(ssem, 16)

        @block.scalar
        def _(scalar: bass.BassEngine):
            scalar.wait_ge(msem, 1)
            ones_level(scalar, 512, 3, 1)
            scalar.wait_ge(msem, 2)
            ones_1024_half(scalar, 1, 0)
            ones_1024_half(scalar, 1, 512)
            ones_2048_quarter(scalar, 1024)
            ones_2048_quarter(scalar, 1536)

        @block.gpsimd
        def _(gpsimd: bass.BassGpSimd):
            # Corner gather first: nothing else touches rows/cols 3968+.
            gpsimd.dma_start(out=corner_in[:, :], in_=corner_dram).then_inc(gsem, 16)
            # The corner chain runs before any bulk DMA is issued: the
            # engine-level wait before affine_select drains ALL outstanding
            # gpsimd DMAs, so running it now (only the gather in flight)
            # keeps the scatter off the kernel tail.
            gpsimd.wait_ge(gsem, 16)
            # iota(p, c) = p - c; keep x where >= 0 (at/below diagonal).
            gpsimd.affine_select(
                out=corner_out[:, :],
                in_=corner_in[:, :],
                pattern=[[-1, P]],
                base=0,
                channel_multiplier=1,
                compare_op=mybir.AluOpType.is_ge,
                fill=1.0,
            )
            gpsimd.dma_start(out=corner_dram, in_=corner_out[:, :]).then_inc(ssem, 16)
            gpsimd.wait_ge(msem, 1)
            # Sub-2KiB-descriptor work lives on SWDGE: it packs small
            # descriptors into 4 KiB packets (HWDGE does not), and queues
            # share SDMA packet slots about equally, so per-queue packet
            # counts are equalized.  SWDGE's share of the 512-level is
            # written in 1 KiB-descriptor form (column halves): 2 KiB
            # descriptors on SWDGE spawn ~4x more tiny bookkeeping packets.
            for k in (1, 2):
                for c0 in (0, 256):
                    gpsimd.dma_start(
                        out=bass.AP(
                            out, 512 + k * 1024 * (S + 1) + c0, [[S, 512], [1, 256]]
                        ),
                        in_=ones[:, :1024],
                    ).then_inc(dsem, 16)
            ones_level(gpsimd, 128, 15, 1)
            band(gpsimd, 3840, P, P)
            gpsimd.wait_ge(msem, 2)
            ones_level(gpsimd, 256, 0, 8)
            band(gpsimd, 0, 1920, 256)
            band(gpsimd, 1920, 1920, 256)

    _cached_nc = nc
    return nc


def _sharded_fn(nc):
    """Build the 8-core PJRT launcher with the output buffer donated.

    Mirrors concourse.bass2jax.run_bass_via_pjrt's multi-core path, except
    the donated output staging buffer is caller-provided (we stage x, the
    in-place masked_fill source) instead of zeros.
    """
    import jax
    from concourse import bass2jax as b2j

    b2j.install_neuronx_cc_hook()

    partition_name = nc.partition_id_tensor.name if nc.partition_id_tensor else None
    in_names: list = []
    out_names: list = []
    out_avals: list = []
    for alloc in nc.m.functions[0].allocations:
        if not isinstance(alloc, mybir.MemoryLocationSet):
            continue
        name = alloc.memorylocations[0].name
        if alloc.kind == "ExternalInput":
            if name != partition_name:
                in_names.append(name)
        elif alloc.kind == "ExternalOutput":
            assert alloc.tensor_shape is not None and alloc.dtype is not None
            out_names.append(name)
            out_avals.append(
                jax.core.ShapedArray(tuple(alloc.tensor_shape), mybir.dt.np(alloc.dtype))
            )
    assert in_names == [] and out_names == ["out"], (in_names, out_names)
    all_in_names = tuple(in_names + out_names + ([partition_name] if partition_name else []))

    def _body(out_buf):
        operands = [out_buf]
        if partition_name is not None:
            operands.append(b2j.partition_id_tensor())
        outs = b2j._bass_exec_p.bind(
            *operands,
            out_avals=tuple(out_avals),
            in_names=all_in_names,
            out_names=tuple(out_names),
            lowering_input_output_aliases=(),
            sim_require_finite=True,
            sim_require_nnan=True,
            nc=nc,
        )
        return tuple(outs)

    devices = jax.devices()[:N_CORES]
    assert len(devices) == N_CORES, f"need {N_CORES} devices, got {len(devices)}"
    mesh = b2j.Mesh(np.asarray(devices), ("core",))
    spec = (b2j.PartitionSpec("core"),)
    return jax.jit(
        b2j.shard_map(_body, mesh=mesh, in_specs=spec, out_specs=spec, check_rep=False),
        donate_argnums=(0,),
        keep_unused=True,
    )


_cached_fn = None


def _run(x_full: np.ndarray, trace: bool = False):
    global _cached_fn
    nc = _build()
    if _cached_fn is None:
        _cached_fn = _sharded_fn(nc)

    x_full = np.ascontiguousarray(np.asarray(x_full, dtype=np.float32))
    staged = x_full.reshape(N_CORES * S, S)

    if not trace:
        out = _cached_fn(staged)[0]
        return np.asarray(out).reshape(N_CORES, S, S), None

    # Profiling path (test harness only): capture core 0's NTFF via the
    # axon hook and run the stock NTFF -> perfetto pipeline.
    from antenv.axon_hooks import get_axon_ntff_profile_hook
    import gauge.profiler
    from concourse import bass_utils
    from concourse._compat import FishPath

    hook = get_axon_ntff_profile_hook()
    neff_dir = tempfile.mkdtemp()
    with hook(neff_dir, [0]):
        out = _cached_fn(staged)[0]
    result = np.asarray(out).reshape(N_CORES, S, S)

    if not glob.glob(os.path.join(neff_dir, "*_body*.ntff")):
        return result, bass_utils.BassKernelResults(
            results=[], instructions_and_trace=None, profile_json=None,
            exec_time_ns=None,
        )
    sharepath = bass_utils.upload_artifacts(neff_dir)
    profile = gauge.profiler.Profile(
        profile_path=FishPath(neff_dir),
        kernel_dev_mode=True,
        profile_on_exit=False,
        bass_kernel=nc.m,
        offline_processing=True,
        fname="*_body*",
        metadata={"artifacts_path": sharepath},
    )
    res = bass_utils._process_ntff_profile(
        profile, neff_dir, nc, list(range(N_CORES)), [0], False, {}, False
    )
    return result, res.as_bass_kernel_results([])


def kernel(x: np.ndarray) -> np.ndarray:
    out, _ = _run(x, trace=False)
    return out
